# revision 40
# baseline (speedup 1.0000x reference)
"""Trainium2 Bass kernel for a DGL-style InteractionNetwork (GNN message passing).

Strategy (edge-parallel, zero collectives, zero device-side gathers):
  * Host sorts edges by receiver. Each of the 8 cores owns a contiguous
    12,500-node range and exactly the edges whose receiver falls in it, so the
    segment-sum is core-local. Edges are packed into 128-edge slices grouped
    by 128-node receiver block, with a per-block slice count Sb[b] shared
    across cores (cross-core max) to minimize padding.
  * Host ALSO gathers node_feat[receivers] and node_feat[senders] into dense
    per-edge tensors (pure data layout, like the sort/pad prep), so the
    device never does an indirect gather:
      c1 = [edge_feat ; nf_recv]      [128, EPAD] bf16
      c2 = [nf_send ; ones]           [65,  EPAD] fp8 (weights stay bf16)
      rb = local receiver index       [128, TS]   f32 (pad slots = 200)
  * Device, per 128-edge slice (4 slices batched per PSUM bank):
      oh  = is_equal(iota, rb)  on Vector  ->  [128e, 128n] one-hot, bf16
      ph  = c1_sliceT @ [We1_e; We1_r]  (+)  c2_sliceT @ [We1_s; be1]
      hid = relu(ph)  on Scalar
      hagg[:, blk] += hidT @ oh  (PSUM accumulate over the block's slices,
                                  4 blocks per [HID, 512] PSUM bank)
  * Node MLP (bf16) interleaved every 4 blocks:
      p1 = wh1T@hagg + wn1nT@nfloc + cdegT@deg;  out = relu(p1+bn1)@Wn2 + bn2
      with wh1 = We2 @ Wn1[:64] folded, cdeg = be2 @ Wn1[:64], deg from host.
  * Output written bf16; host casts and transposes back to [100000, 64] f32.
"""

import numpy as np
import ml_dtypes

BF = ml_dtypes.bfloat16
F8 = ml_dtypes.float8_e4m3

N_NODES = 100000
N_EDGES = 1000000
D = 64
HID = 128
CORES = 8
NLOC = N_NODES // CORES            # 12500
BLK = 128
NBLK = (NLOC + BLK - 1) // BLK     # 98
NLOC_PAD = NBLK * BLK              # 12544
PB = 4                             # blocks per grouped DMA load
CCH = 512                          # node-MLP chunk width

_prog_cache = {}


def _build(Sb):
    import concourse.mybir as mybir
    import concourse.tile as tile
    import concourse.bass as bass
    from concourse import bacc

    bf16 = mybir.dt.bfloat16
    f32 = mybir.dt.float32
    fp8 = mybir.dt.float8e4
    Relu = mybir.ActivationFunctionType.Relu
    Ident = mybir.ActivationFunctionType.Identity
    Max = mybir.AluOpType.max
    IsEq = mybir.AluOpType.is_equal

    Q = [0]
    for s in Sb:
        Q.append(Q[-1] + s)
    TS = Q[-1]
    EPAD = TS * 128
    PCOL = max(sum(Sb[g:g + PB]) * 128 for g in range(0, NBLK, PB))

    nc = bacc.Bacc("TRN2", target_bir_lowering=False, debug=False,
                   num_devices=CORES)

    c1_d = nc.dram_tensor("c1", [128, EPAD], bf16, kind="ExternalInput")
    c2_d = nc.dram_tensor("c2", [65, EPAD], fp8, kind="ExternalInput")
    rb_d = nc.dram_tensor("rb", [128, TS], f32, kind="ExternalInput")
    io4_d = nc.dram_tensor("io4", [128, 512], bf16, kind="ExternalInput")
    wa_d = nc.dram_tensor("wa", [128, HID], bf16, kind="ExternalInput")
    wb_d = nc.dram_tensor("wb", [65, HID], bf16, kind="ExternalInput")
    wh1_d = nc.dram_tensor("wh1", [HID, HID], bf16, kind="ExternalInput")
    wn1n_d = nc.dram_tensor("wn1n", [64, HID], bf16, kind="ExternalInput")
    cdeg_d = nc.dram_tensor("cdeg", [1, HID], bf16, kind="ExternalInput")
    bn1_d = nc.dram_tensor("bn1c", [HID, 1], f32, kind="ExternalInput")
    wn2_d = nc.dram_tensor("wn2", [HID, D], bf16, kind="ExternalInput")
    bn2_d = nc.dram_tensor("bn2c", [D, 1], f32, kind="ExternalInput")
    nfl_d = nc.dram_tensor("nfl", [64, NLOC_PAD], bf16, kind="ExternalInput")
    deg_d = nc.dram_tensor("deg", [1, NLOC_PAD], bf16, kind="ExternalInput")
    out_d = nc.dram_tensor("out_t", [64, NLOC_PAD], bf16, kind="ExternalOutput")

    NQmax = (max(Sb) + 3) // 4     # max 4-slice quads per block

    with tile.TileContext(nc) as tc:
        with tc.tile_pool(name="const", bufs=1) as cp, \
             tc.tile_pool(name="big", bufs=3) as bp, \
             tc.tile_pool(name="hidp", bufs=2 * NQmax + 2) as hp, \
             tc.tile_pool(name="ohp", bufs=2 * NQmax + 2) as op_, \
             tc.tile_pool(name="work", bufs=3) as wp, \
             tc.tile_pool(name="psB", bufs=4, space="PSUM") as psB, \
             tc.tile_pool(name="psH", bufs=2, space="PSUM") as psH, \
             tc.tile_pool(name="psC", bufs=1, space="PSUM") as psC, \
             tc.tile_pool(name="psO", bufs=1, space="PSUM") as psO:

            def cload(d, shape, dtype, tag):
                t = cp.tile(shape, dtype, tag=tag)
                nc.sync.dma_start(t[:], d[:])
                return t

            # critical-path consts first, then the first input pair, then
            # everything only needed later (first emit_C is ~4 blocks in)
            wa = cload(wa_d, [128, HID], bf16, "wa")
            wb = cload(wb_d, [65, HID], bf16, "wb")
            io4 = cload(io4_d, [128, 512], bf16, "io4")
            rbt = cload(rb_d, [128, TS], f32, "rbt")

            pc0 = sum(Sb[0:PB]) * 128
            c1t0 = bp.tile([128, PCOL], bf16, tag="c1t")
            nc.sync.dma_start(c1t0[:, :pc0], c1_d[:, :pc0])
            c2t0 = bp.tile([65, PCOL], fp8, tag="c2t")
            nc.sync.dma_start(c2t0[:, :pc0], c2_d[:, :pc0])

            wh1 = cload(wh1_d, [HID, HID], bf16, "wh1")
            wn1n = cload(wn1n_d, [64, HID], bf16, "wn1n")
            cdeg = cload(cdeg_d, [1, HID], bf16, "cdeg")
            bn1 = cload(bn1_d, [HID, 1], f32, "bn1")
            wn2 = cload(wn2_d, [HID, D], bf16, "wn2")
            bn2 = cload(bn2_d, [D, 1], f32, "bn2")
            nfl = cload(nfl_d, [64, NLOC_PAD], bf16, "nfl")
            deg = cload(deg_d, [1, NLOC_PAD], bf16, "deg")

            hagg = cp.tile([HID, NLOC_PAD], bf16, tag="hagg")

            binfo = {}

            def emit_C(ci, cn):
                n0 = ci * CCH
                p1 = psC.tile([HID, CCH], f32, tag="p1")
                nc.tensor.matmul(out=p1[:, :cn], lhsT=wh1[:],
                                 rhs=hagg[:, n0:n0 + cn],
                                 start=True, stop=False)
                nc.tensor.matmul(out=p1[:, :cn], lhsT=wn1n[:],
                                 rhs=nfl[:, n0:n0 + cn],
                                 start=False, stop=False)
                nc.tensor.matmul(out=p1[:, :cn], lhsT=cdeg[:],
                                 rhs=deg[:, n0:n0 + cn],
                                 start=False, stop=True)
                nh = wp.tile([HID, CCH], bf16, tag="nh")
                nc.scalar.activation(out=nh[:, :cn], in_=p1[:, :cn],
                                     func=Relu, bias=bn1[:, 0:1])
                po = psO.tile([D, CCH], f32, tag="po")
                nc.tensor.matmul(out=po[:, :cn], lhsT=wn2[:], rhs=nh[:, :cn],
                                 start=True, stop=True)
                oc = wp.tile([D, CCH], bf16, tag="oc")
                nc.scalar.activation(out=oc[:, :cn], in_=po[:, :cn],
                                     func=Ident, bias=bn2[:, 0:1])
                nc.sync.dma_start(out_d[:, n0:n0 + cn], oc[:, :cn])

            pagg_cur = [None]

            def emit_aggs(bb):
                ohqs, hidqs = binfo.pop(bb)
                if bb % 4 == 0:
                    pagg_cur[0] = psH.tile([HID, 4 * BLK], f32, tag="pagg",
                                           name="pagg")
                pagg = pagg_cur[0]
                a0 = (bb % 4) * BLK
                Sblk = Sb[bb]
                for s in range(Sblk):
                    nc.tensor.matmul(out=pagg[:, a0:a0 + BLK],
                                     lhsT=hidqs[s // 4][:, (s % 4) * 128:
                                                        (s % 4 + 1) * 128],
                                     rhs=ohqs[s // 4][:, (s % 4) * 128:
                                                      (s % 4 + 1) * 128],
                                     start=(s == 0), stop=(s == Sblk - 1))
                if (bb + 1) % 4 == 0 or bb == NBLK - 1:
                    g = bb // 4
                    gw = (bb % 4 + 1) * BLK
                    nc.vector.tensor_copy(
                        out=hagg[:, g * 512:g * 512 + gw], in_=pagg[:, :gw])
                    emit_C(g, min(CCH, NLOC_PAD - g * CCH))

            gq = 0
            for b in range(NBLK):
                if b % PB == 0:
                    if b == 0:
                        c1t, c2t = c1t0, c2t0
                    else:
                        col0 = Q[b] * 128
                        pcol = sum(Sb[b:b + PB]) * 128
                        c1t = bp.tile([128, PCOL], bf16, tag="c1t")
                        nc.sync.dma_start(c1t[:, :pcol],
                                          c1_d[:, col0:col0 + pcol])
                        c2t = bp.tile([65, PCOL], fp8, tag="c2t")
                        nc.sync.dma_start(c2t[:, :pcol],
                                          c2_d[:, col0:col0 + pcol])
                S = Sb[b]
                base = (Q[b] - Q[b - b % PB]) * 128
                t0 = Q[b]
                hidqs = []
                ohqs = []
                for q in range((S + 3) // 4):
                    qn = min(4, S - q * 4)           # slices in this quad
                    qw = qn * 128
                    ohq = op_.tile([128, 4 * BLK], bf16, tag="ohq")
                    for i in range(qn):
                        t = t0 + q * 4 + i
                        nc.vector.tensor_scalar(
                            out=ohq[:, i * 128:(i + 1) * 128],
                            in0=io4[:, :128],
                            scalar1=rbt[:, t:t + 1],
                            scalar2=None, op0=IsEq)
                    ohqs.append(ohq)
                    ph4 = psB.tile([128, 4 * HID], f32, tag="ph")
                    for i in range(qn):
                        col = base + (q * 4 + i) * 128
                        o = ph4[:, i * HID:(i + 1) * HID]
                        nc.tensor.matmul(out=o, lhsT=c1t[:, col:col + 128],
                                         rhs=wa[:], start=True, stop=False)
                        nc.tensor.matmul(out=o, lhsT=c2t[:, col:col + 128],
                                         rhs=wb[:], start=False, stop=True)
                    hidq = hp.tile([128, 4 * HID], bf16, tag="hid")
                    nc.scalar.activation(out=hidq[:, :qw],
                                         in_=ph4[:, :qw], func=Relu)
                    gq += 1
                    hidqs.append(hidq)
                binfo[b] = (ohqs, hidqs)
                if b >= 1:
                    emit_aggs(b - 1)
            emit_aggs(NBLK - 1)

    nc.compile()
    return nc


def _host_prep(inputs):
    nf = np.ascontiguousarray(np.asarray(inputs["node_feat"], dtype=np.float32))
    ef = np.ascontiguousarray(np.asarray(inputs["edge_feat"], dtype=np.float32))
    snd = np.asarray(inputs["senders"]).astype(np.int64)
    rcv = np.asarray(inputs["receivers"]).astype(np.int64)
    We1 = np.asarray(inputs["We1"], dtype=np.float32)
    be1 = np.asarray(inputs["be1"], dtype=np.float32)
    We2 = np.asarray(inputs["We2"], dtype=np.float32)
    be2 = np.asarray(inputs["be2"], dtype=np.float32)
    Wn1 = np.asarray(inputs["Wn1"], dtype=np.float32)
    bn1 = np.asarray(inputs["bn1"], dtype=np.float32)
    Wn2 = np.asarray(inputs["Wn2"], dtype=np.float32)
    bn2 = np.asarray(inputs["bn2"], dtype=np.float32)

    perm = np.argsort(rcv, kind="stable")
    rs = rcv[perm]
    ss = snd[perm]
    ef_b = ef[perm].astype(BF)
    nf_b = nf.astype(BF)
    nf_8 = nf.astype(F8)

    bounds = np.searchsorted(rs, np.arange(CORES + 1) * NLOC)

    cnt_max = np.zeros(NBLK, dtype=np.int64)
    core_meta = []
    for c in range(CORES):
        lo, hi = int(bounds[c]), int(bounds[c + 1])
        r_loc = (rs[lo:hi] - c * NLOC).astype(np.int64)
        blk = r_loc >> 7
        cnts = np.bincount(blk, minlength=NBLK)
        cnt_max = np.maximum(cnt_max, cnts)
        core_meta.append((lo, hi, r_loc, blk, cnts))

    Sb = np.maximum(1, -(-cnt_max // 128))           # per-block slices
    Qarr = np.zeros(NBLK, dtype=np.int64)
    Qarr[1:] = np.cumsum(Sb)[:-1]
    TS = int(Sb.sum())
    EPAD = TS * 128

    wa = np.ascontiguousarray(We1[0:128]).astype(BF)
    wb = np.concatenate([We1[128:192], be1[None, :]], axis=0).astype(BF)
    wh1 = np.ascontiguousarray(We2 @ Wn1[:64]).astype(BF)
    wn1n = np.ascontiguousarray(Wn1[64:128]).astype(BF)
    cdeg = np.ascontiguousarray((be2 @ Wn1[:64])[None, :]).astype(BF)
    bn1c = np.ascontiguousarray(bn1[:, None]).astype(np.float32)
    wn2 = np.ascontiguousarray(Wn2).astype(BF)
    bn2c = np.ascontiguousarray(bn2[:, None]).astype(np.float32)
    deg_full = np.bincount(rcv, minlength=N_NODES).astype(np.float32)
    io4f = np.ascontiguousarray(
        np.broadcast_to(np.tile(np.arange(128, dtype=np.float32), 4)[None, :],
                        (128, 512)))
    io4 = io4f.astype(BF)

    in_maps = []
    for c in range(CORES):
        lo, hi, r_loc, blk, cnts = core_meta[c]
        ne = hi - lo
        starts = np.zeros(NBLK, dtype=np.int64)
        starts[1:] = np.cumsum(cnts)[:-1]
        within = np.arange(ne, dtype=np.int64) - starts[blk]
        slot = Qarr[blk] * 128 + within           # = t*128 + p
        t_idx = Qarr[blk] + (within >> 7)
        p_idx = within & 127
        rloc_in_blk = r_loc - (blk << 7)

        c1r = np.zeros((EPAD, 128), dtype=BF)
        c1r[slot, 0:64] = ef_b[lo:hi]
        c1r[slot, 64:128] = nf_b[rs[lo:hi]]
        c1 = np.ascontiguousarray(c1r.T)

        c2r = np.zeros((EPAD, 65), dtype=F8)
        c2r[slot, 0:64] = nf_8[ss[lo:hi]]
        c2r[slot, 64] = 1.0
        c2 = np.ascontiguousarray(c2r.T)

        rb = np.full((128, TS), 200.0, dtype=np.float32)
        rb[p_idx, t_idx] = rloc_in_blk.astype(np.float32)

        nfl = np.zeros((64, NLOC_PAD), dtype=BF)
        nfl[:, :NLOC] = nf_b[c * NLOC:(c + 1) * NLOC].T

        deg = np.zeros((1, NLOC_PAD), dtype=BF)
        deg[0, :NLOC] = deg_full[c * NLOC:(c + 1) * NLOC].astype(BF)

        in_maps.append({
            "c1": c1, "c2": c2, "rb": rb, "io4": io4,
            "wa": wa, "wb": wb, "wh1": wh1, "wn1n": wn1n, "cdeg": cdeg,
            "bn1c": bn1c, "wn2": wn2, "bn2c": bn2c,
            "nfl": nfl, "deg": deg,
        })
    return tuple(int(x) for x in Sb), in_maps


def _run(inputs, trace=False):
    from concourse.bass_utils import run_bass_kernel_spmd

    Sb, in_maps = _host_prep(inputs)
    if Sb not in _prog_cache:
        _prog_cache[Sb] = _build(Sb)
    nc = _prog_cache[Sb]
    res = run_bass_kernel_spmd(nc, in_maps, core_ids=list(range(CORES)),
                               trace=trace)
    out = np.empty((N_NODES, D), dtype=np.float32)
    for c in range(CORES):
        out[c * NLOC:(c + 1) * NLOC] = \
            np.asarray(res.results[c]["out_t"])[:, :NLOC].T.astype(np.float32)
    return out, res


def kernel(**inputs):
    out, _ = _run(inputs, trace=False)
    return out
